# revision 4
# baseline (speedup 1.0000x reference)
"""EquiAttention Trainium2 kernel.

Computes the reference nn_EquiAttention forward pass on 8 NeuronCores,
data-parallel over the batch axis (64 batches -> 8 per core).

Math refactoring (validated on CPU, see baseline docstring for the
original folding):
  q = [vecs.flat (64) | scalars (64)] (128-dim), kT = BD.T @ qT with
  BD = blockdiag(metric-G, H); scores fold to a 128-dim contraction.
  NEW vs baseline: the per-key bias c2.s_m = bq_s.(Wk_s s_m) is folded
  into the embedding by shifting the scalar inputs on the host:
  scal' = scal + d with Wq_s d = bq_s. Then
    (Wq_s(s_q+d)).(Wk_s(s_m+d)) = (Wq_s s_q + bq_s).(Wk_s s_m) + f(q)
  and f(q) is softmax-invariant. V needs no exp(c2.s) weighting: the
  denominator comes from the exp's accum_out.

Device structure per batch (per core):
  - hi/lo fp16 split of the normalized [vec|scal] embedding is done in
    token-major layout, then transposed to emb-major via the DMA xbar
    (2 fp16 [128,1024] block-transposes; replaces 8 PE transposes + 2
    PSUM evacuations per batch of the baseline).
  - kT = blockdiag.T @ qT 3-pass fp16 from PSUM as before.
  - scores per 128-query block land in ONE 2-bank PSUM tile [128,1024];
    row-max is a single DVE reduce (negate=True), P = exp(S-max) is a
    single ACT op whose accum_out yields the softmax denominator free.
  - P^T via DMA xbar, two query blocks per DMA (4KB runs/partition);
    all xbar transposes on the nc.sync HWDGE queue only.
  - PV: accT[64, 512] += Vaug[mc].T @ P^T chunks per query half, with
    Vaug = plain E-projection of the vecs (fp16), PE-transposed back,
    scaled by reciprocal denominators, one output DMA per half.
"""

import numpy as np

B, N = 64, 1024
NCORES = 8
BL = B // NCORES          # batches per core
NB = N // 128             # 128-row blocks per sequence
SCALE = 1.0 / np.sqrt(192.0)

_CACHE = {}


def _build_program():
    import concourse.bacc as bacc
    import concourse.tile as tile
    from concourse import mybir

    f32 = mybir.dt.float32

    nc = bacc.Bacc("TRN2", target_bir_lowering=False,
                   debug=False, num_devices=NCORES)

    aps = {
        "vectors": nc.dram_tensor("vectors", [BL, N, 64], f32,
                                  kind="ExternalInput").ap(),
        "scalars": nc.dram_tensor("scalars", [BL, N, 64], f32,
                                  kind="ExternalInput").ap(),
        "BD": nc.dram_tensor("BD", [128, 128], f32, kind="ExternalInput").ap(),
        "WvE": nc.dram_tensor("WvE", [128, 64], f32, kind="ExternalInput").ap(),
        "out": nc.dram_tensor("out", [BL, N, 64], f32, kind="ExternalOutput").ap(),
    }

    with tile.TileContext(nc) as tc:
        _emit(tc, aps)

    nc.compile()
    return nc


def _emit(tc, aps):
    from contextlib import ExitStack
    import concourse.bass as bass
    import concourse.masks as masks
    from concourse import mybir

    nc = tc.nc
    f32 = mybir.dt.float32
    f16 = mybir.dt.float16
    PS = "PSUM"
    Act = mybir.ActivationFunctionType
    Alu = mybir.AluOpType
    X = mybir.AxisListType.X

    vecs_d, scal_d = aps["vectors"], aps["scalars"]
    bd_d, wve_d, out_d = aps["BD"], aps["WvE"], aps["out"]

    with ExitStack() as ctx:
        singles = ctx.enter_context(tc.tile_pool(name="singles", bufs=1))
        raw = ctx.enter_context(tc.tile_pool(name="raw", bufs=2))
        emb = ctx.enter_context(tc.tile_pool(name="emb", bufs=2))
        small = ctx.enter_context(tc.tile_pool(name="small", bufs=6))
        dens = ctx.enter_context(tc.tile_pool(name="dens", bufs=2))
        pP = ctx.enter_context(tc.tile_pool(name="pP", bufs=3))
        pPT = ctx.enter_context(tc.tile_pool(name="pPT", bufs=2))
        outp = ctx.enter_context(tc.tile_pool(name="outp", bufs=4))
        accsb = ctx.enter_context(tc.tile_pool(name="accsb", bufs=4))
        psS = ctx.enter_context(tc.tile_pool(name="psS", bufs=2, space=PS))
        psAcc = ctx.enter_context(tc.tile_pool(name="psAcc", bufs=1, space=PS))
        psMisc = ctx.enter_context(tc.tile_pool(name="psMisc", bufs=2, space=PS))

        ident = singles.tile([128, 128], f32)
        masks.make_identity(nc, ident[:])
        bd = singles.tile([128, 128], f32)
        nc.gpsimd.dma_start(out=bd[:], in_=bd_d[:, :])
        bdhi = singles.tile([128, 128], f16)
        nc.vector.tensor_copy(bdhi[:], bd[:])
        bdlo = singles.tile([128, 128], f16)
        nc.vector.tensor_sub(bdlo[:], bd[:], bdhi[:])
        wve16 = singles.tile([128, 64], f16)
        nc.gpsimd.dma_start(out=wve16[:], in_=wve_d[:, :].bitcast(f32))

        def embed_pre(b):
            # ---------- embedding: DMA + normalize (token-major) ----------
            vs = raw.tile([128, NB, 128], f32, tag="vs")
            nc.gpsimd.dma_start(out=vs[:, :, 64:128],
                                in_=scal_d[b].rearrange("(c p) f -> p c f", p=128))
            vraw = raw.tile([128, NB, 64], f32, tag="vraw")
            nc.gpsimd.dma_start(out=vraw[:],
                                in_=vecs_d[b].rearrange("(c p) f -> p c f", p=128))

            # Lorentz normalization of the 16 four-vectors per particle
            sq = raw.tile([128, NB, 16, 4], f32, tag="sq")
            nc.scalar.activation(out=sq[:], in_=vraw[:], func=Act.Square)
            nrm = raw.tile([128, NB, 16], f32, tag="nrm")
            nc.vector.tensor_sub(nrm[:], sq[:, :, :, 0], sq[:, :, :, 1])
            nc.vector.tensor_sub(nrm[:], nrm[:], sq[:, :, :, 2])
            nc.vector.tensor_sub(nrm[:], nrm[:], sq[:, :, :, 3])
            nc.scalar.activation(out=nrm[:], in_=nrm[:], func=Act.Abs)
            nc.vector.tensor_scalar_max(nrm[:], nrm[:], 1e-5)
            nc.scalar.activation(out=nrm[:], in_=nrm[:], func=Act.Sqrt)
            rn = raw.tile([128, NB, 16], f32, tag="rn")
            nc.vector.reciprocal(rn[:], nrm[:])

            # vs[:, :, 0:64] = vraw * rn  (rn broadcast over the 4-vector
            # components via a zero-stride AP dim)
            rn_b = bass.AP(tensor=rn.tensor, offset=rn.offset,
                           ap=[rn.ap[0], [rn.ap[1][0], NB], rn.ap[2], [0, 4]])
            nc.vector.tensor_mul(
                vs[:, :, 0:64].rearrange("p c (j k) -> p c j k", k=4),
                vraw[:].rearrange("p c (j k) -> p c j k", k=4), rn_b)
            return vs

        def embed_hilo(vs):
            # hi/lo fp16 split in token-major, then xbar-transpose to
            # emb-major; all xbar DMAs share the nc.sync queue
            vshi = raw.tile([128, NB, 128], f16, tag="vshi")
            nc.vector.tensor_copy(vshi[:], vs[:])
            vslo = raw.tile([128, NB, 128], f16, tag="vslo")
            nc.vector.tensor_sub(vslo[:], vs[:], vshi[:])
            qhi = emb.tile([128, NB, 128], f16, tag="qhi")
            nc.sync.dma_start_transpose(
                qhi[:], vshi[:].rearrange("p c e -> p (c e)"))
            qlo = emb.tile([128, NB, 128], f16, tag="qlo")
            nc.sync.dma_start_transpose(
                qlo[:], vslo[:].rearrange("p c e -> p (c e)"))
            return qhi, qlo

        def embed_pe(qhi, qlo):
            # kT = blockdiag(G~, H~).T @ qT (3-pass fp16), hi/lo from PSUM
            khi = emb.tile([128, NB, 128], f16, tag="khi")
            klo = emb.tile([128, NB, 128], f16, tag="klo")
            for hh in range(2):
                cs = slice(hh * 4, (hh + 1) * 4)
                pk = psMisc.tile([128, 512], f32, tag="misc")
                qhi_h = qhi[:, cs].rearrange("p c e -> p (c e)")
                qlo_h = qlo[:, cs].rearrange("p c e -> p (c e)")
                nc.tensor.matmul(pk[:], bdhi[:], qhi_h,
                                 start=True, stop=False)
                nc.tensor.matmul(pk[:], bdhi[:], qlo_h,
                                 start=False, stop=False)
                nc.tensor.matmul(pk[:], bdlo[:], qhi_h,
                                 start=False, stop=True)
                nc.scalar.copy(khi[:, cs].rearrange("p c e -> p (c e)"), pk[:])
                nc.vector.tensor_sub(
                    klo[:, cs].rearrange("p c e -> p (c e)"), pk[:],
                    khi[:, cs].rearrange("p c e -> p (c e)"))

            # Vaug[mc] = E-projection of normalized vecs (fp16, no bias)
            vaug = emb.tile([128, NB, 64], f16, tag="vaug")
            for hh in range(2):
                pvt = psMisc.tile([128, 4, 64], f32, tag="misc")
                for j in range(4):
                    nc.tensor.matmul(pvt[:, j], qhi[:, hh * 4 + j], wve16[:],
                                     start=True, stop=True)
                nc.scalar.copy(
                    vaug[:, hh * 4:(hh + 1) * 4].rearrange("p c e -> p (c e)"),
                    pvt[:].rearrange("p c e -> p (c e)"))
            return qhi, qlo, khi, klo, vaug

        def attn_qblocks(emb_tiles, den):
            qhi, qlo, khi, klo, vaug = emb_tiles
            # P^T layout: ptf[p, qb, mc, q'] = P[qb*128+q', mc*128+p]
            ptf = pPT.tile([128, NB, NB, 128], f16, tag="ptf")

            def q_block(qb, P2):
                S = psS.tile([128, 2, 512], f32, tag="S")
                kh = khi[:].rearrange("p c e -> p (c e)")
                kl = klo[:].rearrange("p c e -> p (c e)")
                # lhsT=qhi once for 4 passes, then qlo for 2 (fewer LDW)
                for h in range(2):
                    cols = slice(h * 512, (h + 1) * 512)
                    nc.tensor.matmul(S[:, h], qhi[:, qb], kh[:, cols],
                                     start=True, stop=False)
                    nc.tensor.matmul(S[:, h], qhi[:, qb], kl[:, cols],
                                     start=False, stop=False)
                for h in range(2):
                    cols = slice(h * 512, (h + 1) * 512)
                    nc.tensor.matmul(S[:, h], qlo[:, qb], kh[:, cols],
                                     start=False, stop=True)
                negmax = small.tile([128, 1], f32, tag="negmax")
                nc.vector.tensor_reduce(
                    negmax[:], S[:].rearrange("p a b -> p (a b)"), axis=X,
                    op=Alu.max, negate=True)
                if P2 is None:
                    P2 = pP.tile([128, 2, N], f16, tag="P")
                nc.scalar.activation(
                    out=P2[:, qb % 2], in_=S[:].rearrange("p a b -> p (a b)"),
                    func=Act.Exp, bias=negmax[:], scale=1.0,
                    accum_out=den[:, qb:qb + 1])
                if qb % 2 == 1:
                    # two query blocks per xbar transpose: 4KB contiguous
                    # runs per partition; single HWDGE queue for all xbars
                    nc.sync.dma_start_transpose(
                        ptf[:, qb - 1:qb + 1],
                        P2[:].rearrange("p two m -> p (two m)"))
                return P2

            P2 = None
            for qb in range(NB):
                P2 = q_block(qb, P2)
                if qb % 2 == 1:
                    P2 = None
            return ptf

        def attn_pv_epi(b, emb_tiles, ptf, den):
            qhi, qlo, khi, klo, vaug = emb_tiles
            rden = small.tile([128, NB], f32, tag="rden")
            nc.vector.reciprocal(rden[:], den[:])

            # accT[64, qhalf] += Vaug[mc].T @ P^T[mc] chunks; vaug weights
            # shared across the two query halves per chunk
            accs = [psAcc.tile([64, 512], f32, tag=f"accT{h}", name=f"accT{h}")
                    for h in range(2)]
            for mc in range(NB):
                for hh in range(2):
                    nc.tensor.matmul(accs[hh][:], vaug[:, mc],
                                     ptf[:, hh * 4:(hh + 1) * 4, mc, :],
                                     start=(mc == 0), stop=(mc == NB - 1))

            for hh in range(2):
                accsb_t = accsb.tile([64, 512], f32, tag="accsb")
                nc.scalar.copy(accsb_t[:], accs[hh][:])
                ot = psMisc.tile([128, 4, 64], f32, tag="misc")
                for j in range(4):
                    nc.tensor.transpose(ot[:, j], accsb_t[:, j * 128:(j + 1) * 128],
                                        ident[0:64, 0:64])
                ob = outp.tile([128, 4, 64], f32, tag="ob")
                for j in range(4):
                    nc.vector.tensor_scalar_mul(
                        ob[:, j], ot[:, j], rden[:, hh * 4 + j:hh * 4 + j + 1])
                nc.gpsimd.dma_start(
                    out=out_d[b, hh * 512:(hh + 1) * 512, :]
                    .rearrange("(j p) f -> p j f", p=128),
                    in_=ob[:])

        def embed(b):
            vs = embed_pre(b)
            return embed_pe(*embed_hilo(vs))

        # One-batch-ahead software pipelining: embed(b+1) is emitted
        # (and thus prioritized) before attention(b).
        prev = embed(0)
        for b in range(1, BL):
            cur = embed(b)
            den = dens.tile([128, NB], f32, tag="den")
            attn_pv_epi(b - 1, prev, attn_qblocks(prev, den), den)
            prev = cur
        den = dens.tile([128, NB], f32, tag="den")
        attn_pv_epi(BL - 1, prev, attn_qblocks(prev, den), den)


def _host_weights(Wq, Wk, Wv, Wq_s, Wk_s, bq_s):
    """Fold the tiny EquiLinear weights (float64 precompute, cast f32)."""
    METRIC = np.array([1.0, -1.0, -1.0, -1.0], dtype=np.float64)
    G = Wq.astype(np.float64).T @ Wk.astype(np.float64)            # [16,16]
    BD = np.zeros((128, 128), dtype=np.float64)
    for k in range(4):
        # lhsT[(j',k), (j,k)] = SCALE * METRIC[k] * G[j, j']
        BD[k:64:4, k:64:4] = SCALE * METRIC[k] * G.T
    # lhsT[h, g] = SCALE * H[g, h],  H = Wq_s.T @ Wk_s
    BD[64:, 64:] = SCALE * (Wk_s.astype(np.float64).T @ Wq_s.astype(np.float64))
    E = np.exp(Wv.astype(np.float64))                              # [16,16]
    WvE = np.zeros((128, 64), dtype=np.float64)
    for k in range(4):
        # rhs[(j,k), (i,k)] = E[i, j]
        WvE[k:64:4, k:64:4] = E.T
    # scalar-bias fold: shift d with Wq_s d = bq_s
    d = np.linalg.solve(Wq_s.astype(np.float64), bq_s.astype(np.float64))
    return (np.ascontiguousarray(BD, dtype=np.float32),
            np.ascontiguousarray(WvE, dtype=np.float32),
            d)


def _prepare_in_maps(vectors, scalars, Wq, Wq_s, bq_s, Wk, Wk_s, bk_s, Wv):
    BD, WvE, d = _host_weights(Wq, Wk, Wv, Wq_s, Wk_s, bq_s)
    vecs_flat = np.ascontiguousarray(
        np.asarray(vectors).reshape(B, N, 64), dtype=np.float32)
    scal = (np.asarray(scalars, dtype=np.float64) + d).astype(np.float32)

    in_maps = []
    for c in range(NCORES):
        sl = slice(c * BL, (c + 1) * BL)
        in_maps.append({
            "vectors": np.ascontiguousarray(vecs_flat[sl]),
            "scalars": np.ascontiguousarray(scal[sl]),
            "BD": BD,
            "WvE": WvE,
        })
    return in_maps


def _run(in_maps, **kw):
    from concourse.bass_utils import run_bass_kernel_spmd
    nc = _get_program()
    return run_bass_kernel_spmd(nc, in_maps, list(range(NCORES)), **kw)


def _get_program():
    if "nc" not in _CACHE:
        _CACHE["nc"] = _build_program()
    return _CACHE["nc"]


def kernel(vectors, scalars, Wq, Wq_s, bq_s, Wk, Wk_s, bk_s, Wv):
    args = [np.asarray(a, dtype=np.float32) for a in
            (vectors, scalars, Wq, Wq_s, bq_s, Wk, Wk_s, bk_s, Wv)]
    in_maps = _prepare_in_maps(*args)
    res = _run(in_maps)
    out = np.concatenate([res.results[c]["out"] for c in range(NCORES)], axis=0)
    return out.reshape(B, N, 16, 4).astype(np.float32)


# revision 6
# speedup vs baseline: 1.0240x; 1.0240x over previous
"""EquiAttention Trainium2 kernel.

Computes the reference nn_EquiAttention forward pass on 8 NeuronCores,
data-parallel over the batch axis (64 batches -> 8 per core).

Math refactoring (validated on CPU):
  q = [vecs.flat (64) | scalars (64)] (128-dim), kT = BD.T @ qT with
  BD = blockdiag(metric-G, H); scores fold to a 128-dim contraction.
  The per-key bias c2.s_m = bq_s.(Wk_s s_m) is folded into the embedding
  by shifting the scalar inputs on the host: scal' = scal + d with
  Wq_s d = bq_s; the remaining terms are per-query constants that
  softmax drops. V needs no exp(c2.s) weighting; the denominator comes
  from a constant ones column in Vaug.

Device structure per batch (per core):
  - Lorentz norm chain runs on GpSimd with rn = exp(-0.25*ln(nrm^2))
    on ACT: the only ACT functions in the kernel are Ln/Exp/Copy (one
    table set), avoiding per-batch activation-table reloads.
  - hi/lo fp16 split of the normalized [vec|scal] embedding is done in
    token-major layout, then transposed to emb-major via the DMA xbar
    (2 fp16 [128,1024] block-transposes; replaces 8 PE transposes + 2
    PSUM evacuations per batch). Emitted AFTER the attention xbars of
    the previous batch: the nc.sync HWDGE queue is FIFO and embed
    xbars at the head would stall the P^T transposes.
  - scores per 128-query block land in ONE 2-bank PSUM tile [128,1024];
    row-max is a single DVE reduce (negate=True), P = exp(S-max) is a
    single ACT op; 3-pass fp16 hi/lo scores as in the baseline.
  - P^T via DMA xbar, two query blocks per DMA (4KB runs/partition);
    all xbar transposes on the nc.sync HWDGE queue only.
  - PV: accT[65, 512] += Vaug[mc].T @ P^T chunks, one query half at a
    time (half 0 only needs the first two xbars -> starts mid-attention),
    PE-transposed back, normalized, one output DMA per half.
"""

import numpy as np

B, N = 64, 1024
NCORES = 8
BL = B // NCORES          # batches per core
NB = N // 128             # 128-row blocks per sequence
SCALE = 1.0 / np.sqrt(192.0)

_CACHE = {}


def _build_program():
    import concourse.bacc as bacc
    import concourse.tile as tile
    from concourse import mybir

    f32 = mybir.dt.float32

    nc = bacc.Bacc("TRN2", target_bir_lowering=False,
                   debug=False, num_devices=NCORES)

    aps = {
        "vectors": nc.dram_tensor("vectors", [BL, N, 64], f32,
                                  kind="ExternalInput").ap(),
        "scalars": nc.dram_tensor("scalars", [BL, N, 64], f32,
                                  kind="ExternalInput").ap(),
        "BD": nc.dram_tensor("BD", [128, 128], f32, kind="ExternalInput").ap(),
        "WvE": nc.dram_tensor("WvE", [128, 64], f32, kind="ExternalInput").ap(),
        "out": nc.dram_tensor("out", [BL, N, 64], f32, kind="ExternalOutput").ap(),
    }

    with tile.TileContext(nc) as tc:
        _emit(tc, aps)

    nc.compile()
    return nc


def _emit(tc, aps):
    from contextlib import ExitStack
    import concourse.bass as bass
    import concourse.masks as masks
    from concourse import mybir

    nc = tc.nc
    f32 = mybir.dt.float32
    f16 = mybir.dt.float16
    PS = "PSUM"
    Act = mybir.ActivationFunctionType
    Alu = mybir.AluOpType
    X = mybir.AxisListType.X

    vecs_d, scal_d = aps["vectors"], aps["scalars"]
    bd_d, wve_d, out_d = aps["BD"], aps["WvE"], aps["out"]

    with ExitStack() as ctx:
        singles = ctx.enter_context(tc.tile_pool(name="singles", bufs=1))
        raw = ctx.enter_context(tc.tile_pool(name="raw", bufs=2))
        emb = ctx.enter_context(tc.tile_pool(name="emb", bufs=2))
        small = ctx.enter_context(tc.tile_pool(name="small", bufs=6))
        pP = ctx.enter_context(tc.tile_pool(name="pP", bufs=3))
        pPT = ctx.enter_context(tc.tile_pool(name="pPT", bufs=2))
        outp = ctx.enter_context(tc.tile_pool(name="outp", bufs=4))
        accsb = ctx.enter_context(tc.tile_pool(name="accsb", bufs=4))
        psS = ctx.enter_context(tc.tile_pool(name="psS", bufs=2, space=PS))
        psAcc = ctx.enter_context(tc.tile_pool(name="psAcc", bufs=1, space=PS))
        psMisc = ctx.enter_context(tc.tile_pool(name="psMisc", bufs=2, space=PS))

        ident = singles.tile([128, 128], f32)
        masks.make_identity(nc, ident[:])
        bd = singles.tile([128, 128], f32)
        nc.gpsimd.dma_start(out=bd[:], in_=bd_d[:, :])
        bdhi = singles.tile([128, 128], f16)
        nc.vector.tensor_copy(bdhi[:], bd[:])
        bdlo = singles.tile([128, 128], f16)
        nc.vector.tensor_sub(bdlo[:], bd[:], bdhi[:])
        wve16 = singles.tile([128, 64], f16)
        nc.gpsimd.dma_start(out=wve16[:], in_=wve_d[:, :].bitcast(f32))

        def embed_pre(b):
            # ---------- embedding: DMA + normalize (token-major) ----------
            vs = raw.tile([128, NB, 128], f32, tag="vs")
            nc.gpsimd.dma_start(out=vs[:, :, 64:128],
                                in_=scal_d[b].rearrange("(c p) f -> p c f", p=128))
            vraw = raw.tile([128, NB, 64], f32, tag="vraw")
            nc.gpsimd.dma_start(out=vraw[:],
                                in_=vecs_d[b].rearrange("(c p) f -> p c f", p=128))

            # Lorentz normalization: GpSimd combines, rn = exp(-ln(nrm^2)/4)
            sq = raw.tile([128, NB, 16, 4], f32, tag="sq")
            nc.gpsimd.tensor_mul(sq[:], vraw[:].rearrange("p c (j k) -> p c j k", k=4),
                                 vraw[:].rearrange("p c (j k) -> p c j k", k=4))
            nrm = raw.tile([128, NB, 16], f32, tag="nrm")
            nc.gpsimd.tensor_sub(nrm[:], sq[:, :, :, 0], sq[:, :, :, 1])
            nc.gpsimd.tensor_sub(nrm[:], nrm[:], sq[:, :, :, 2])
            nc.gpsimd.tensor_sub(nrm[:], nrm[:], sq[:, :, :, 3])
            nc.gpsimd.tensor_mul(nrm[:], nrm[:], nrm[:])
            nc.gpsimd.tensor_scalar_max(nrm[:], nrm[:], 1e-10)
            rn = raw.tile([128, NB, 16], f32, tag="rn")
            nc.scalar.activation(out=rn[:], in_=nrm[:], func=Act.Ln)
            nc.scalar.activation(out=rn[:], in_=rn[:], func=Act.Exp, scale=-0.25)

            # vs[:, :, 0:64] = vraw * rn  (rn broadcast over the 4-vector
            # components via a zero-stride AP dim)
            rn_b = bass.AP(tensor=rn.tensor, offset=rn.offset,
                           ap=[rn.ap[0], [rn.ap[1][0], NB], rn.ap[2], [0, 4]])
            nc.vector.tensor_mul(
                vs[:, :, 0:64].rearrange("p c (j k) -> p c j k", k=4),
                vraw[:].rearrange("p c (j k) -> p c j k", k=4), rn_b)
            return vs

        def embed_hilo(vs):
            # hi/lo fp16 split in token-major, then xbar-transpose to
            # emb-major; all xbar DMAs share the nc.sync queue
            vshi = raw.tile([128, NB, 128], f16, tag="vshi")
            nc.vector.tensor_copy(vshi[:], vs[:])
            vslo = raw.tile([128, NB, 128], f16, tag="vslo")
            nc.gpsimd.tensor_sub(vslo[:], vs[:], vshi[:])
            qhi = emb.tile([128, NB, 128], f16, tag="qhi")
            nc.sync.dma_start_transpose(
                qhi[:], vshi[:].rearrange("p c e -> p (c e)"))
            qlo = emb.tile([128, NB, 128], f16, tag="qlo")
            nc.sync.dma_start_transpose(
                qlo[:], vslo[:].rearrange("p c e -> p (c e)"))
            return qhi, qlo

        def embed_pe(qhi, qlo, write_ones):
            # kT = blockdiag(G~, H~).T @ qT (3-pass fp16), hi/lo from PSUM
            khi = emb.tile([128, NB, 128], f16, tag="khi")
            klo = emb.tile([128, NB, 128], f16, tag="klo")
            for hh in range(2):
                cs = slice(hh * 4, (hh + 1) * 4)
                pk = psMisc.tile([128, 512], f32, tag="misc")
                qhi_h = qhi[:, cs].rearrange("p c e -> p (c e)")
                qlo_h = qlo[:, cs].rearrange("p c e -> p (c e)")
                nc.tensor.matmul(pk[:], bdhi[:], qhi_h,
                                 start=True, stop=False)
                nc.tensor.matmul(pk[:], bdhi[:], qlo_h,
                                 start=False, stop=False)
                nc.tensor.matmul(pk[:], bdlo[:], qhi_h,
                                 start=False, stop=True)
                nc.scalar.copy(khi[:, cs].rearrange("p c e -> p (c e)"), pk[:])
                nc.vector.tensor_sub(
                    klo[:, cs].rearrange("p c e -> p (c e)"), pk[:],
                    khi[:, cs].rearrange("p c e -> p (c e)"))

            # Vaug[mc] = [E-projection of normalized vecs (fp16) | ones]
            vaug = emb.tile([128, NB, 65], f16, tag="vaug")
            for hh in range(2):
                pvt = psMisc.tile([128, 4, 64], f32, tag="misc")
                for j in range(4):
                    nc.tensor.matmul(pvt[:, j], qhi[:, hh * 4 + j], wve16[:],
                                     start=True, stop=True)
                nc.scalar.copy(vaug[:, hh * 4:(hh + 1) * 4, 0:64], pvt[:])
            if write_ones:
                # ones column persists in the (round-robin) pool buffer
                nc.vector.memset(vaug[:, :, 64], 1.0)
            return qhi, qlo, khi, klo, vaug

        def attn_qblocks(emb_tiles):
            qhi, qlo, khi, klo, vaug = emb_tiles
            # P^T layout: ptf[p, qb, mc, q'] = P[qb*128+q', mc*128+p]
            ptf = pPT.tile([128, NB, NB, 128], f16, tag="ptf")

            def q_block(qb, P2):
                S = psS.tile([128, 2, 512], f32, tag="S")
                kh = khi[:].rearrange("p c e -> p (c e)")
                kl = klo[:].rearrange("p c e -> p (c e)")
                # lhsT=qhi once for 4 passes, then qlo for 2 (fewer LDW)
                for h in range(2):
                    cols = slice(h * 512, (h + 1) * 512)
                    nc.tensor.matmul(S[:, h], qhi[:, qb], kh[:, cols],
                                     start=True, stop=False)
                    nc.tensor.matmul(S[:, h], qhi[:, qb], kl[:, cols],
                                     start=False, stop=False)
                for h in range(2):
                    cols = slice(h * 512, (h + 1) * 512)
                    nc.tensor.matmul(S[:, h], qlo[:, qb], kh[:, cols],
                                     start=False, stop=True)
                negmax = small.tile([128, 1], f32, tag="negmax")
                nc.vector.tensor_reduce(
                    negmax[:], S[:].rearrange("p a b -> p (a b)"), axis=X,
                    op=Alu.max, negate=True)
                if P2 is None:
                    P2 = pP.tile([128, 2, N], f16, tag="P")
                nc.scalar.activation(
                    out=P2[:, qb % 2], in_=S[:].rearrange("p a b -> p (a b)"),
                    func=Act.Exp, bias=negmax[:], scale=1.0)
                if qb % 2 == 1:
                    # two query blocks per xbar transpose: 4KB contiguous
                    # runs per partition; single HWDGE queue for all xbars
                    nc.sync.dma_start_transpose(
                        ptf[:, qb - 1:qb + 1],
                        P2[:].rearrange("p two m -> p (two m)"))
                return P2

            P2 = None
            for qb in range(NB):
                P2 = q_block(qb, P2)
                if qb % 2 == 1:
                    P2 = None
            return ptf

        def attn_pv_epi(b, emb_tiles, ptf):
            qhi, qlo, khi, klo, vaug = emb_tiles
            # accT[65, qhalf] += Vaug[mc].T @ P^T[mc]; half hh only needs
            # xbars hh*2..hh*2+1 -> half 0 starts mid-attention
            for hh in range(2):
                accT = psAcc.tile([65, 512], f32, tag="accT")
                for mc in range(NB):
                    nc.tensor.matmul(accT[:], vaug[:, mc],
                                     ptf[:, hh * 4:(hh + 1) * 4, mc, :],
                                     start=(mc == 0), stop=(mc == NB - 1))
                accsb_t = accsb.tile([65, 512], f32, tag="accsb")
                nc.scalar.copy(accsb_t[:], accT[:])
                ot = psMisc.tile([128, 4, 65], f32, tag="misc")
                for j in range(4):
                    nc.tensor.transpose(ot[:, j], accsb_t[:, j * 128:(j + 1) * 128],
                                        ident[0:65, 0:65])
                rden = small.tile([128, 4], f32, tag="rden")
                nc.vector.reciprocal(rden[:], ot[:, :, 64])
                ob = outp.tile([128, 4, 64], f32, tag="ob")
                for j in range(4):
                    nc.vector.tensor_scalar_mul(ob[:, j], ot[:, j, 0:64],
                                                rden[:, j:j + 1])
                nc.gpsimd.dma_start(
                    out=out_d[b, hh * 512:(hh + 1) * 512, :]
                    .rearrange("(j p) f -> p j f", p=128),
                    in_=ob[:])

        # Software pipelining; embed xbars for b are emitted AFTER the
        # attention xbars for b-1 (FIFO queue head-of-line avoidance).
        prev = embed_pe(*embed_hilo(embed_pre(0)), True)
        for b in range(1, BL):
            vs = embed_pre(b)
            ptf = attn_qblocks(prev)
            cur = embed_pe(*embed_hilo(vs), b < 2)
            attn_pv_epi(b - 1, prev, ptf)
            prev = cur
        attn_pv_epi(BL - 1, prev, attn_qblocks(prev))


def _host_weights(Wq, Wk, Wv, Wq_s, Wk_s, bq_s):
    """Fold the tiny EquiLinear weights (float64 precompute, cast f32)."""
    METRIC = np.array([1.0, -1.0, -1.0, -1.0], dtype=np.float64)
    G = Wq.astype(np.float64).T @ Wk.astype(np.float64)            # [16,16]
    BD = np.zeros((128, 128), dtype=np.float64)
    for k in range(4):
        # lhsT[(j',k), (j,k)] = SCALE * METRIC[k] * G[j, j']
        BD[k:64:4, k:64:4] = SCALE * METRIC[k] * G.T
    # lhsT[h, g] = SCALE * H[g, h],  H = Wq_s.T @ Wk_s
    BD[64:, 64:] = SCALE * (Wk_s.astype(np.float64).T @ Wq_s.astype(np.float64))
    E = np.exp(Wv.astype(np.float64))                              # [16,16]
    WvE = np.zeros((128, 64), dtype=np.float64)
    for k in range(4):
        # rhs[(j,k), (i,k)] = E[i, j]
        WvE[k:64:4, k:64:4] = E.T
    # scalar-bias fold: shift d with Wq_s d = bq_s
    d = np.linalg.solve(Wq_s.astype(np.float64), bq_s.astype(np.float64))
    return (np.ascontiguousarray(BD, dtype=np.float32),
            np.ascontiguousarray(WvE, dtype=np.float32),
            d)


def _prepare_in_maps(vectors, scalars, Wq, Wq_s, bq_s, Wk, Wk_s, bk_s, Wv):
    BD, WvE, d = _host_weights(Wq, Wk, Wv, Wq_s, Wk_s, bq_s)
    vecs_flat = np.ascontiguousarray(
        np.asarray(vectors).reshape(B, N, 64), dtype=np.float32)
    scal = (np.asarray(scalars, dtype=np.float64) + d).astype(np.float32)

    in_maps = []
    for c in range(NCORES):
        sl = slice(c * BL, (c + 1) * BL)
        in_maps.append({
            "vectors": np.ascontiguousarray(vecs_flat[sl]),
            "scalars": np.ascontiguousarray(scal[sl]),
            "BD": BD,
            "WvE": WvE,
        })
    return in_maps


def _run(in_maps, **kw):
    from concourse.bass_utils import run_bass_kernel_spmd
    nc = _get_program()
    return run_bass_kernel_spmd(nc, in_maps, list(range(NCORES)), **kw)


def _get_program():
    if "nc" not in _CACHE:
        _CACHE["nc"] = _build_program()
    return _CACHE["nc"]


def kernel(vectors, scalars, Wq, Wq_s, bq_s, Wk, Wk_s, bk_s, Wv):
    args = [np.asarray(a, dtype=np.float32) for a in
            (vectors, scalars, Wq, Wq_s, bq_s, Wk, Wk_s, bk_s, Wv)]
    in_maps = _prepare_in_maps(*args)
    res = _run(in_maps)
    out = np.concatenate([res.results[c]["out"] for c in range(NCORES)], axis=0)
    return out.reshape(B, N, 16, 4).astype(np.float32)
